# revision 1
# baseline (speedup 1.0000x reference)
"""CondConv2d (moe_routing) Trainium2 Bass kernel.

Full-input contract: kernel(**inputs) takes the unsharded inputs
  x      (32, 256, 56, 56) f32
  weight (2048, 256, 3, 3) f32   -- expert bank, (E*COUT, CIN, 3, 3), E=8
  fc_w   (8, 256) f32
  fc_b   (8,) f32
and returns the full (32, 256, 56, 56) f32 output of

  gate = sigmoid(mean_hw(x) @ fc_w.T + fc_b)              # (n, 8)
  w    = (gate @ weight.reshape(8, -1)).reshape(n, 256, 256, 3, 3)
  out[s] = conv2d(x[s], w[s], padding=1)

Sharding: data-parallel over batch across 8 NeuronCores (4 samples/core),
expert bank + fc params replicated.

Per-core program (heavy matmuls in float32r = full PE rate):
  phase A  gating: x loaded flat, reduce_sum over H*W, tiny PE matmul with
           fc_w^T, sigmoid on ACT (scale=1/3136 folds the mean), gates
           broadcast across partitions via a DRAM roundtrip.
  phase B  expert mixing ON the PE with an expert-interleaved contraction:
           bank rows are loaded as K-tiles whose 128 partitions are
           (e, j) = 8 experts x 16 output-channel lanes, so ONE matmul per
           16-channel group both sums over all 8 experts and transposes the
           bank into the (ci, co) lhsT layout the conv needs:
             out[ci, (s, co')] = sum_(e,j) bank[(e,co),ci] * R[(e,j),(s,c')]
             R[(e,j), (s,c')]  = g[s,e] * delta(j, c')       (N=64 matmuls)
  phase C  3x3 conv as 18 accumulating matmuls (2 ci tiles x 9 taps) per
           448-column PSUM chunk (8 output rows) over a zero-padded 58-wide
           image layout in SBUF.
"""

import numpy as np

import concourse.bass as bass
import concourse.mybir as mybir
import concourse.tile as tile
from concourse import bacc
from concourse.bass_utils import run_bass_kernel_spmd
from concourse.masks import make_identity

# Problem constants (hardcoded per contract).
N_FULL = 32
NCORES = 8
NS = N_FULL // NCORES  # 4 samples per core
E = 8
CIN = 256
COUT = 256
K = 3
H = W = 56
PW = W + 2  # padded row width 58
HW = H * W  # 3136
CH = 8 * W  # 448 output columns per PSUM chunk (8 rows x 56 cols)
NCHUNK = H // 8  # 7
FPAD = PW * (H + 2)  # 3364 padded-x free size
CIT = CIN // 128  # 2 contraction tiles
COT = COUT // 128  # 2 output-partition tiles

f32 = mybir.dt.float32
f32r = mybir.dt.float32r

_CACHED_NC = None


def _build(repeat: int = 1):
    nc = bacc.Bacc(trn_type="TRN2", target_bir_lowering=False, debug=False)

    x_d = nc.dram_tensor("x", (NS, CIN, H, W), f32, kind="ExternalInput").ap()
    w_d = nc.dram_tensor(
        "weight", (E * COUT, CIN, K, K), f32, kind="ExternalInput"
    ).ap()
    fcw_d = nc.dram_tensor("fc_w", (E, CIN), f32, kind="ExternalInput").ap()
    fcb_d = nc.dram_tensor("fc_b", (E,), f32, kind="ExternalInput").ap()
    out_d = nc.dram_tensor("out", (NS, COUT, H, W), f32, kind="ExternalOutput").ap()

    # bank viewed as (e, co, ci*9): row (e,co), 2304 wide
    bank = w_d.rearrange("r c h w -> r (c h w)")  # (2048, 2304)
    # tiled 16x16 identity constant: tI[p, c] = 1 iff p % 16 == c
    tI_d = nc.inline_tensor(
        np.tile(np.eye(16, dtype=np.float32), (8, 1)), name="tiled_eye16"
    ).ap()

    with tile.TileContext(nc) as tc:
      for _rep in range(repeat):
        with (
            tc.tile_pool(name="smalls", bufs=1) as smalls,
            tc.tile_pool(name="wmix", bufs=COT * CIT * 9) as wmpool,
            tc.tile_pool(name="dram", bufs=1, space="DRAM") as dramp,
            tc.tile_pool(name="ps", bufs=8, space="PSUM") as pp,
            tc.tile_pool(name="xpad", bufs=2 * CIT) as xpp,
            tc.tile_pool(name="outp", bufs=2) as op,
        ):
            # ---------------- phase A: gating ----------------
            means = []
            for ci_t in range(CIT):
                means.append(
                    smalls.tile([128, NS], f32, tag=f"means{ci_t}", name=f"means{ci_t}")
                )
            fcwt = []
            for ci_t in range(CIT):
                t = smalls.tile([128, E], f32, tag=f"fcwt{ci_t}", name=f"fcwt{ci_t}")
                # fc_w^T slice: (ci in tile, e) -- tiny strided DMA
                nc.sync.dma_start(
                    out=t,
                    in_=fcw_d.rearrange("e c -> c e")[
                        ci_t * 128 : (ci_t + 1) * 128, :
                    ],
                )
                fcwt.append(t)
            fcb_sb = smalls.tile([E, 1], f32, tag="fcb")
            nc.sync.dma_start(out=fcb_sb, in_=fcb_d.unsqueeze(1))
            tI_sb = smalls.tile([128, 16], f32, tag="tI")
            nc.sync.dma_start(out=tI_sb, in_=tI_d)

            with tc.tile_pool(name="xflat", bufs=2) as xfp:
                for s in range(NS):
                    for ci_t in range(CIT):
                        xt = xfp.tile([128, HW], f32, tag="xflat", name="xflat")
                        nc.sync.dma_start(
                            out=xt,
                            in_=x_d[s, ci_t * 128 : (ci_t + 1) * 128].rearrange(
                                "c h w -> c (h w)"
                            ),
                        )
                        nc.vector.reduce_sum(
                            out=means[ci_t][:, s : s + 1],
                            in_=xt[:],
                            axis=mybir.AxisListType.X,
                        )

            # logits[e, s] = sum_ci fc_w[e, ci] * xsum[ci, s]
            ps_g = pp.tile([E, NS], f32, tag="ps", name="ps_g")
            for ci_t in range(CIT):
                nc.tensor.matmul(
                    ps_g[:],
                    fcwt[ci_t][:],
                    means[ci_t][:],
                    start=(ci_t == 0),
                    stop=(ci_t == CIT - 1),
                )
            gate_sb = smalls.tile([E, NS], f32, tag="gate")
            # gate = sigmoid(logits / (H*W) + fc_b)
            nc.scalar.activation(
                gate_sb[:],
                ps_g[:],
                mybir.ActivationFunctionType.Sigmoid,
                bias=fcb_sb[:],
                scale=1.0 / float(HW),
            )
            # gate roundtrip: gv_s[p] = g[s, p//16]  (per-partition scalars)
            gdram = dramp.tile([E, NS], f32, tag="gd", name="gdram")
            nc.sync.dma_start(out=gdram, in_=gate_sb)
            gvs = []
            for s in range(NS):
                gv = smalls.tile([128, 1], f32, tag=f"gv{s}", name=f"gv{s}")
                src = bass.AP(
                    tensor=gdram.tensor,
                    offset=gdram.offset + s,
                    ap=[[NS, E], [0, 16]],
                )
                nc.sync.dma_start(out=gv[:], in_=src)
                gvs.append(gv)

            # R[(e,j), (s,c')] = g[s,e] * delta(j, c')
            rt = smalls.tile([128, NS * 16], f32r, tag="rt")
            for s in range(NS):
                nc.vector.tensor_scalar_mul(
                    rt[:, s * 16 : (s + 1) * 16], tI_sb[:], gvs[s][:]
                )

            # ---------------- phase B: expert mixing on PE ----------------
            # wm[(co_t, ci_t, tap)][ci_l, s*128 + co_l] =
            #     sum_e g[s,e] * weight[e, co_t*128+co_l, ci_t*128+ci_l, tap]
            wm = {}
            with tc.tile_pool(name="bank", bufs=10) as bkp:
                for co_t in range(COT):
                    for ci_t in range(CIT):
                        bts = []
                        for cbl in range(8):
                            # K-tile rows: (e, j) -> bank row e*256 + co_t*128
                            #                         + cbl*16 + j
                            bt = bkp.tile([128, 128, 9], f32r, tag="bank", name="bank")
                            src = bass.AP(
                                tensor=bank.tensor,
                                offset=bank.offset
                                + (co_t * 128 + cbl * 16) * 2304
                                + ci_t * 1152,
                                ap=[[256 * 2304, E], [2304, 16], [1, 1152]],
                            )
                            nc.gpsimd.dma_start(
                                out=bt[:].rearrange("p a b -> p (a b)"),
                                in_=src,
                            )
                            bts.append(bt)
                        for tap in range(9):
                            ps_m = pp.tile(
                                [128, NS, 128], f32, tag="ps", name="ps_m"
                            )
                            for cbl in range(8):
                                nc.tensor.matmul(
                                    ps_m[:, :, cbl * 16 : (cbl + 1) * 16],
                                    bts[cbl][:, :, tap],
                                    rt[:],
                                    start=True,
                                    stop=True,
                                )
                            wt = wmpool.tile([128, NS * 128], f32r, tag="wm", name="wm")
                            nc.scalar.copy(
                                wt[:], ps_m[:].rearrange("p a b -> p (a b)")
                            )
                            wm[(co_t, ci_t, tap)] = wt

            # ---------------- phase C: conv ----------------
            if True:
                xts_all = {}
                for s in range(NS):
                    for ci_t in range(CIT):
                        xt = xpp.tile([128, FPAD], f32r, tag="xpad", name="xpad")
                        xf = xt[:].bitcast(f32)
                        # zero only the halo: top row, bottom row, and the
                        # (col 57, col 0) pairs between consecutive rows
                        nc.vector.memset(xf[:, 0:PW], 0.0)
                        nc.vector.memset(xf[:, FPAD - PW : FPAD], 0.0)
                        pairs = xf[:, PW - 1 : PW - 1 + (H + 1) * PW].rearrange(
                            "p (a b) -> p a b", b=PW
                        )[:, :, 0:2]
                        nc.vector.memset(pairs, 0.0)
                        nc.gpsimd.dma_start(
                            out=xt[:]
                            .rearrange("p (h w) -> p h w", w=PW)[
                                :, 1 : H + 1, 1 : W + 1
                            ],
                            in_=x_d[s, ci_t * 128 : (ci_t + 1) * 128],
                        )
                        xts_all[(s, ci_t)] = xt
                for s in range(NS):
                    xts = [xts_all[(s, 0)], xts_all[(s, 1)]]
                    for co_t in range(COT):
                        ot = op.tile([128, HW], f32, tag="outp", name="outp")
                        for c in range(NCHUNK):
                            ps_c = pp.tile([128, CH], f32, tag="ps", name="ps_c")
                            i = 0
                            for ci_t in range(CIT):
                                xv = xts[ci_t][:].rearrange("p (h w) -> p h w", w=PW)
                                for kh in range(K):
                                    for kw in range(K):
                                        tap = kh * K + kw
                                        rhs = xv[
                                            :, 8 * c + kh : 8 * c + kh + 8, kw : kw + W
                                        ]
                                        nc.tensor.matmul(
                                            ps_c[:],
                                            wm[(co_t, ci_t, tap)][
                                                :, s * 128 : (s + 1) * 128
                                            ],
                                            rhs,
                                            start=(i == 0),
                                            stop=(i == CIT * 9 - 1),
                                        )
                                        i += 1
                            nc.scalar.copy(ot[:, c * CH : (c + 1) * CH], ps_c[:])
                            if c == 3:
                                nc.sync.dma_start(
                                    out=out_d[s, co_t * 128 : (co_t + 1) * 128, 0:32],
                                    in_=ot[:, 0 : 4 * CH].rearrange(
                                        "p (h w) -> p h w", w=W
                                    ),
                                )
                        nc.sync.dma_start(
                            out=out_d[s, co_t * 128 : (co_t + 1) * 128, 32:H],
                            in_=ot[:, 4 * CH : HW].rearrange(
                                "p (h w) -> p h w", w=W
                            ),
                        )

    nc.compile()
    return nc


def _get_nc():
    global _CACHED_NC
    if _CACHED_NC is None:
        _CACHED_NC = _build()
    return _CACHED_NC


def kernel(x, weight, fc_w, fc_b):
    assert x.shape == (N_FULL, CIN, H, W), x.shape
    assert weight.shape == (E * COUT, CIN, K, K), weight.shape
    x = np.ascontiguousarray(x, dtype=np.float32)
    weight = np.ascontiguousarray(weight, dtype=np.float32)
    fc_w = np.ascontiguousarray(fc_w, dtype=np.float32)
    fc_b = np.ascontiguousarray(fc_b, dtype=np.float32)

    nc = _get_nc()
    in_maps = [
        {
            "x": np.ascontiguousarray(x[i * NS : (i + 1) * NS]),
            "weight": weight,
            "fc_w": fc_w,
            "fc_b": fc_b,
        }
        for i in range(NCORES)
    ]
    res = run_bass_kernel_spmd(nc, in_maps, core_ids=list(range(NCORES)))
    out = np.concatenate([res.results[i]["out"] for i in range(NCORES)], axis=0)
    return out



# revision 2
# speedup vs baseline: 1.3567x; 1.3567x over previous
"""CondConv2d (moe_routing) Trainium2 Bass kernel, v2 (bf16 pipeline).

Full-input contract: kernel(**inputs) takes the unsharded inputs
  x      (32, 256, 56, 56) f32
  weight (2048, 256, 3, 3) f32   -- expert bank, (E*COUT, CIN, 3, 3), E=8
  fc_w   (8, 256) f32
  fc_b   (8,) f32
and returns the full (32, 256, 56, 56) f32 output of

  gate = sigmoid(mean_hw(x) @ fc_w.T + fc_b)              # (n, 8)
  w    = (gate @ weight.reshape(8, -1)).reshape(n, 256, 256, 3, 3)
  out[s] = conv2d(x[s], w[s], padding=1)

Sharding: data-parallel over batch across 8 NeuronCores (4 samples/core),
expert bank + fc params replicated.

Host pre-pass (layout/dtype only): x and the expert bank are converted to
bf16; the bank is additionally repacked into the exact expert-interleaved
K-tile layout the mixing matmuls consume, so every DMA is a contiguous
line-rate transfer.

Per-core program (heavy matmuls in bf16 = full PE rate + fast weight load):
  phase A  x loaded flat+contiguous (split across both HWDGE queues),
           per-tile reduce_sum for the gating mean on DVE, on-chip pad-copy
           into a zero-haloed 58-wide image layout on GPSIMD. Gating logits
           via tiny PE matmul, sigmoid on ACT (scale=1/3136 folds the mean),
           gates broadcast across partitions with a second tiny PE matmul
           (no DRAM roundtrip).
  phase B  expert mixing ON the PE with an expert-interleaved contraction:
           bank K-tiles have partitions (e, j) = 8 experts x 16 lanes, so one
           matmul per 16-channel group both sums over experts and transposes
           the bank into the (ci, co) lhsT layout the conv needs:
             out[ci, (s, co')] = sum_(e,j) bank[(e,co),ci] * R[(e,j),(s,c')]
             R[(e,j), (s,c')]  = g[s,e] * delta(j, c')
  phase C  3x3 conv as 18 accumulating bf16 matmuls (2 ci tiles x 9 taps)
           per 448-column PSUM chunk (8 output rows), f32 accumulate, DVE
           evacuation, streamed f32 stores. co_t=1's bank load + mixing
           overlap co_t=0's conv.
"""

import numpy as np

import concourse.bass as bass
import concourse.mybir as mybir
import concourse.tile as tile
from concourse import bacc
from concourse.bass_utils import run_bass_kernel_spmd

# Problem constants (hardcoded per contract).
N_FULL = 32
NCORES = 8
NS = N_FULL // NCORES  # 4 samples per core
E = 8
CIN = 256
COUT = 256
K = 3
H = W = 56
PW = W + 2  # padded row width 58
HW = H * W  # 3136
CH = 8 * W  # 448 output columns per PSUM chunk (8 rows x 56 cols)
NCHUNK = H // 8  # 7
FPAD = PW * (H + 2)  # 3364 padded-x free size
CIT = CIN // 128  # 2 contraction tiles
COT = COUT // 128  # 2 output-partition tiles
NTAP = K * K  # 9

f32 = mybir.dt.float32
bf16 = mybir.dt.bfloat16
BF_NP = mybir.dt.np(bf16)

_CACHED_NC = None


def _build():
    nc = bacc.Bacc(trn_type="TRN2", target_bir_lowering=False, debug=False)

    x_d = nc.dram_tensor("xb", (NS, CIN, H, W), bf16, kind="ExternalInput").ap()
    # pre-packed bank: [co_t, ci_t, cbl, (e,j)=128, (ci_l,tap)=1152] bf16
    bank_d = nc.dram_tensor(
        "bankp", (COT, CIT, 8, 128, 128 * NTAP), bf16, kind="ExternalInput"
    ).ap()
    fcw_d = nc.dram_tensor("fc_w", (E, CIN), f32, kind="ExternalInput").ap()
    fcb_d = nc.dram_tensor("fc_b", (E,), f32, kind="ExternalInput").ap()
    out_d = nc.dram_tensor("out", (NS, COUT, H, W), f32, kind="ExternalOutput").ap()

    # tiled 16x16 identity: tI[p, c] = 1 iff p % 16 == c
    tI_d = nc.inline_tensor(
        np.tile(np.eye(16, dtype=np.float32), (8, 1)), name="tiled_eye16"
    ).ap()
    # broadcast selector: S[e, p] = 1 iff p // 16 == e
    S_d = nc.inline_tensor(
        np.repeat(np.eye(E, dtype=np.float32), 16, axis=1), name="bcast8"
    ).ap()

    with tile.TileContext(nc) as tc:
        with (
            tc.tile_pool(name="smalls", bufs=1) as smalls,
            tc.tile_pool(name="wmix", bufs=COT * CIT * NTAP) as wmpool,
            tc.tile_pool(name="ps", bufs=8, space="PSUM") as pp,
            tc.tile_pool(name="xpad", bufs=NS * CIT) as xpp,
            tc.tile_pool(name="xflat", bufs=3) as xfp,
            tc.tile_pool(name="bank", bufs=3) as bkp,
            tc.tile_pool(name="outp", bufs=2) as op,
        ):
            # ---------------- phase A: x load + gating ----------------
            means = []
            fcwt = []
            for ci_t in range(CIT):
                means.append(
                    smalls.tile([128, NS], f32, tag=f"means{ci_t}", name=f"means{ci_t}")
                )
                t = smalls.tile([128, E], f32, tag=f"fcwt{ci_t}", name=f"fcwt{ci_t}")
                nc.scalar.dma_start(
                    out=t,
                    in_=fcw_d.rearrange("e c -> c e")[
                        ci_t * 128 : (ci_t + 1) * 128, :
                    ],
                )
                fcwt.append(t)
            fcb_sb = smalls.tile([E, 1], f32, tag="fcb")
            nc.scalar.dma_start(out=fcb_sb, in_=fcb_d.unsqueeze(1))
            tI_sb = smalls.tile([128, 16], f32, tag="tI")
            nc.scalar.dma_start(out=tI_sb, in_=tI_d)
            S_sb = smalls.tile([E, 128], f32, tag="S")
            nc.scalar.dma_start(out=S_sb, in_=S_d)

            xts_all = {}
            for s in range(NS):
                for ci_t in range(CIT):
                    xf = xfp.tile([128, HW], bf16, tag="xflat", name="xflat")
                    q = nc.sync if ((s * CIT + ci_t) % 2 == 0) else nc.scalar
                    q.dma_start(
                        out=xf,
                        in_=x_d[s, ci_t * 128 : (ci_t + 1) * 128].rearrange(
                            "c h w -> c (h w)"
                        ),
                    )
                    nc.vector.reduce_sum(
                        out=means[ci_t][:, s : s + 1],
                        in_=xf[:],
                        axis=mybir.AxisListType.X,
                    )
                    xt = xpp.tile([128, FPAD], bf16, tag="xpad", name="xpad")
                    # zero the halo: top row, bottom row, (col 57, col 0)
                    # pairs between consecutive rows
                    nc.gpsimd.memset(xt[:, 0:PW], 0.0)
                    nc.gpsimd.memset(xt[:, FPAD - PW : FPAD], 0.0)
                    pairs = xt[:, PW - 1 : PW - 1 + (H + 1) * PW].rearrange(
                        "p (a b) -> p a b", b=PW
                    )[:, :, 0:2]
                    nc.gpsimd.memset(pairs, 0.0)
                    nc.gpsimd.tensor_copy(
                        xt[:]
                        .rearrange("p (h w) -> p h w", w=PW)[:, 1 : H + 1, 1 : W + 1],
                        xf[:].rearrange("p (h w) -> p h w", w=W),
                    )
                    xts_all[(s, ci_t)] = xt

            # logits[e, s] = sum_ci fc_w[e, ci] * xsum[ci, s]
            ps_g = pp.tile([E, NS], f32, tag="ps", name="ps_g")
            for ci_t in range(CIT):
                nc.tensor.matmul(
                    ps_g[:],
                    fcwt[ci_t][:],
                    means[ci_t][:],
                    start=(ci_t == 0),
                    stop=(ci_t == CIT - 1),
                )
            gate_sb = smalls.tile([E, NS], f32, tag="gate")
            # gate = sigmoid(logits / (H*W) + fc_b)
            nc.scalar.activation(
                gate_sb[:],
                ps_g[:],
                mybir.ActivationFunctionType.Sigmoid,
                bias=fcb_sb[:],
                scale=1.0 / float(HW),
            )
            # broadcast gates to all partitions: gb[p, s] = gate[p // 16, s]
            ps_b = pp.tile([128, NS], f32, tag="ps", name="ps_b")
            nc.tensor.matmul(ps_b[:], S_sb[:], gate_sb[:], start=True, stop=True)
            gb_sb = smalls.tile([128, NS], f32, tag="gb")
            nc.scalar.copy(gb_sb[:], ps_b[:])

            # R[(e,j), (s,c')] = g[s,e] * delta(j, c')
            rt = smalls.tile([128, NS * 16], bf16, tag="rt")
            for s in range(NS):
                nc.vector.tensor_scalar_mul(
                    rt[:, s * 16 : (s + 1) * 16], tI_sb[:], gb_sb[:, s : s + 1]
                )

            # ---------------- phases B+C interleaved over co_t ----------------
            def emit_bank_load(co_t, ci_t):
                bt = bkp.tile([128, 8, 128, NTAP], bf16, tag="bank", name="bank")
                for cbl in range(8):
                    nc.scalar.dma_start(
                        out=bt[:, cbl].rearrange("p a b -> p (a b)"),
                        in_=bank_d[co_t, ci_t, cbl],
                    )
                return bt

            def emit_mix(co_t, ci_t, bt, wm):
                # wm[(co_t, ci_t, tap)][ci_l, s*128 + co_l] =
                #     sum_e g[s,e] * weight[e, co_t*128+co_l, ci_t*128+ci_l, tap]
                for tap in range(NTAP):
                    ps_m = pp.tile([128, NS, 128], f32, tag="ps", name="ps_m")
                    for cbl in range(8):
                        nc.tensor.matmul(
                            ps_m[:, :, cbl * 16 : (cbl + 1) * 16],
                            bt[:, cbl, :, tap],
                            rt[:],
                            start=True,
                            stop=True,
                        )
                    wt = wmpool.tile([128, NS * 128], bf16, tag="wm", name="wm")
                    nc.scalar.copy(wt[:], ps_m[:].rearrange("p a b -> p (a b)"))
                    wm[(co_t, ci_t, tap)] = wt

            wm = {}
            bts = {}
            # co_t=0 bank halves first; co_t=1 queued behind them
            for ci_t in range(CIT):
                bts[(0, ci_t)] = emit_bank_load(0, ci_t)
            for ci_t in range(CIT):
                emit_mix(0, ci_t, bts[(0, ci_t)], wm)
            for ci_t in range(CIT):
                bts[(1, ci_t)] = emit_bank_load(1, ci_t)

            for co_t in range(COT):
                if co_t == 1:
                    for ci_t in range(CIT):
                        emit_mix(1, ci_t, bts[(1, ci_t)], wm)
                for s in range(NS):
                    xts = [xts_all[(s, 0)], xts_all[(s, 1)]]
                    ot = op.tile([128, HW], f32, tag="outp", name="outp")
                    for c in range(NCHUNK):
                        ps_c = pp.tile([128, CH], f32, tag="ps", name="ps_c")
                        i = 0
                        for ci_t in range(CIT):
                            xv = xts[ci_t][:].rearrange("p (h w) -> p h w", w=PW)
                            for kh in range(K):
                                for kw in range(K):
                                    tap = kh * K + kw
                                    rhs = xv[
                                        :, 8 * c + kh : 8 * c + kh + 8, kw : kw + W
                                    ]
                                    nc.tensor.matmul(
                                        ps_c[:],
                                        wm[(co_t, ci_t, tap)][
                                            :, s * 128 : (s + 1) * 128
                                        ],
                                        rhs,
                                        start=(i == 0),
                                        stop=(i == CIT * NTAP - 1),
                                    )
                                    i += 1
                        nc.vector.tensor_copy(
                            ot[:, c * CH : (c + 1) * CH], ps_c[:]
                        )
                        if c == 3:
                            nc.sync.dma_start(
                                out=out_d[s, co_t * 128 : (co_t + 1) * 128, 0:32],
                                in_=ot[:, 0 : 4 * CH].rearrange(
                                    "p (h w) -> p h w", w=W
                                ),
                            )
                    nc.sync.dma_start(
                        out=out_d[s, co_t * 128 : (co_t + 1) * 128, 32:H],
                        in_=ot[:, 4 * CH : HW].rearrange("p (h w) -> p h w", w=W),
                    )

    nc.compile()
    return nc


def _get_nc():
    global _CACHED_NC
    if _CACHED_NC is None:
        _CACHED_NC = _build()
    return _CACHED_NC


def _pack_bank(weight):
    # bankp[co_t, ci_t, cbl, e*16+j, ci_l*9+tap] =
    #     weight[e*256 + co_t*128 + cbl*16 + j, ci_t*128 + ci_l, tap]
    w6 = weight.reshape(E, COT, 8, 16, CIT, 128, NTAP)
    return np.ascontiguousarray(
        w6.transpose(1, 4, 2, 0, 3, 5, 6).reshape(COT, CIT, 8, 128, 128 * NTAP)
    ).astype(BF_NP)


def kernel(x, weight, fc_w, fc_b):
    assert x.shape == (N_FULL, CIN, H, W), x.shape
    assert weight.shape == (E * COUT, CIN, K, K), weight.shape
    x = np.ascontiguousarray(x, dtype=np.float32)
    weight = np.ascontiguousarray(weight, dtype=np.float32)
    fc_w = np.ascontiguousarray(fc_w, dtype=np.float32)
    fc_b = np.ascontiguousarray(fc_b, dtype=np.float32)

    xb = x.astype(BF_NP)
    bankp = _pack_bank(weight)

    nc = _get_nc()
    in_maps = [
        {
            "xb": np.ascontiguousarray(xb[i * NS : (i + 1) * NS]),
            "bankp": bankp,
            "fc_w": fc_w,
            "fc_b": fc_b,
        }
        for i in range(NCORES)
    ]
    res = run_bass_kernel_spmd(nc, in_maps, core_ids=list(range(NCORES)))
    out = np.concatenate([res.results[i]["out"] for i in range(NCORES)], axis=0)
    return out


# revision 5
# speedup vs baseline: 1.5139x; 1.1159x over previous
"""CondConv2d (moe_routing) Trainium2 Bass kernel, v3 (bf16, no padding).

Full-input contract: kernel(**inputs) takes the unsharded inputs
  x      (32, 256, 56, 56) f32
  weight (2048, 256, 3, 3) f32   -- expert bank, (E*COUT, CIN, 3, 3), E=8
  fc_w   (8, 256) f32
  fc_b   (8,) f32
and returns the full (32, 256, 56, 56) f32 output of

  gate = sigmoid(mean_hw(x) @ fc_w.T + fc_b)              # (n, 8)
  w    = (gate @ weight.reshape(8, -1)).reshape(n, 256, 256, 3, 3)
  out[s] = conv2d(x[s], w[s], padding=1)

Sharding: data-parallel over batch across 8 NeuronCores (4 samples/core),
expert bank + fc params replicated.

Host pre-pass (layout/dtype only): x and the expert bank are converted to
bf16; the bank is repacked into the expert-interleaved K-tile layout the
mixing matmuls consume (contiguous stationary slices -> FWL fast weight
load), so every DMA is a contiguous line-rate transfer.

Per-core program (heavy matmuls in bf16 = full PE rate):
  phase A  x loaded flat+contiguous, split across both HWDGE queues; the
           bank streams on the gpsimd SWDGE queue concurrently. Gating
           row-sums split across DVE (reduce_sum) and ACT (Copy with
           accum_out). Gating logits via tiny PE matmul, sigmoid on ACT,
           gates broadcast across partitions with a second tiny PE matmul,
           R built by 4 ACT scaled copies.
  phase B  expert mixing ON the PE with an expert-interleaved contraction:
           bank K-tiles have partitions (e, j) = 8 experts x 16 lanes, so
           one matmul per 16-channel group both sums over experts and
           transposes the bank into the (ci, co) lhsT layout the conv
           needs:
             out[ci, (s, co')] = sum_(e,j) bank[(e,co),ci] * R[(e,j),(s,c')]
             R[(e,j), (s,c')]  = g[s,e] * delta(j, c')
  phase C  3x3 conv as 18 accumulating bf16 matmuls (2 ci tiles x 9 taps)
           per 448-column PSUM chunk (8 output rows) DIRECTLY on the flat
           56-wide x tiles: zero-padding is realized by clamping each tap's
           matmul to the valid row/column range (the centre tap goes first
           with start=True, covering the full bank so has_written is set
           everywhere). f32 accumulate, DVE evacuation, streamed f32 stores.
"""

import numpy as np

import concourse.bass as bass
import concourse.mybir as mybir
import concourse.tile as tile
from concourse import bacc
from concourse.bass_utils import run_bass_kernel_spmd

# Problem constants (hardcoded per contract).
N_FULL = 32
NCORES = 8
NS = N_FULL // NCORES  # 4 samples per core
E = 8
CIN = 256
COUT = 256
K = 3
H = W = 56
HW = H * W  # 3136
CH = 8 * W  # 448 output columns per PSUM chunk (8 rows x 56 cols)
NCHUNK = H // 8  # 7
CIT = CIN // 128  # 2 contraction tiles
COT = COUT // 128  # 2 output-partition tiles
NTAP = K * K  # 9

# centre tap first (full range, start=True), then the edge taps
TAP_ORDER = [(1, 1), (0, 0), (0, 1), (0, 2), (1, 0), (1, 2), (2, 0), (2, 1), (2, 2)]

f32 = mybir.dt.float32
bf16 = mybir.dt.bfloat16
BF_NP = mybir.dt.np(bf16)

_CACHED_NC = None


def _build():
    nc = bacc.Bacc(trn_type="TRN2", target_bir_lowering=False, debug=False)

    x_d = nc.dram_tensor("xb", (NS, CIN, H, W), bf16, kind="ExternalInput").ap()
    # pre-packed bank: [co_t, ci_t, (e,j)=128, (tap, cbl, ci_l)=9216] bf16
    bank_d = nc.dram_tensor(
        "bankp", (COT, CIT, 128, NTAP * 8 * 128), bf16, kind="ExternalInput"
    ).ap()
    fcw_d = nc.dram_tensor("fc_w", (E, CIN), f32, kind="ExternalInput").ap()
    fcb_d = nc.dram_tensor("fc_b", (E,), f32, kind="ExternalInput").ap()
    out_d = nc.dram_tensor("out", (NS, COUT, H, W), f32, kind="ExternalOutput").ap()

    # tiled 16x16 identity: tI[p, c] = 1 iff p % 16 == c
    tI_d = nc.inline_tensor(
        np.tile(np.eye(16, dtype=np.float32), (8, 1)), name="tiled_eye16"
    ).ap()
    # broadcast selector: S[e, p] = 1 iff p // 16 == e
    S_d = nc.inline_tensor(
        np.repeat(np.eye(E, dtype=np.float32), 16, axis=1), name="bcast8"
    ).ap()

    with tile.TileContext(nc) as tc:
        with (
            tc.tile_pool(name="smalls", bufs=1) as smalls,
            tc.tile_pool(name="wmix", bufs=COT * CIT * NTAP) as wmpool,
            tc.tile_pool(name="ps", bufs=7, space="PSUM") as pp,
            tc.tile_pool(name="ps0", bufs=1, space="PSUM") as pp0,
            tc.tile_pool(name="xflat", bufs=NS * CIT) as xfp,
            tc.tile_pool(name="bank", bufs=COT * CIT) as bkp,
            tc.tile_pool(name="outp", bufs=2) as op,
        ):
            # ------------- phase A: DMAs first, then gating chain -------------
            means = []
            fcwt = []
            for ci_t in range(CIT):
                means.append(
                    smalls.tile([128, NS], f32, tag=f"means{ci_t}", name=f"means{ci_t}")
                )
                t = smalls.tile([128, E], f32, tag=f"fcwt{ci_t}", name=f"fcwt{ci_t}")
                nc.scalar.dma_start(
                    out=t,
                    in_=fcw_d.rearrange("e c -> c e")[
                        ci_t * 128 : (ci_t + 1) * 128, :
                    ],
                )
                fcwt.append(t)
            fcb_sb = smalls.tile([E, 1], f32, tag="fcb")
            nc.scalar.dma_start(out=fcb_sb, in_=fcb_d.unsqueeze(1))
            tI_sb = smalls.tile([128, 16], f32, tag="tI")
            nc.scalar.dma_start(out=tI_sb, in_=tI_d)
            S_sb = smalls.tile([E, 128], f32, tag="S")
            nc.scalar.dma_start(out=S_sb, in_=S_d)

            # x tiles: flat contiguous loads, alternating HWDGE queues
            xts_all = {}
            for s in range(NS):
                for ci_t in range(CIT):
                    xf = xfp.tile([128, HW], bf16, tag="xflat", name="xflat")
                    q = nc.sync if ((s * CIT + ci_t) % 2 == 0) else nc.scalar
                    q.dma_start(
                        out=xf,
                        in_=x_d[s, ci_t * 128 : (ci_t + 1) * 128].rearrange(
                            "c h w -> c (h w)"
                        ),
                    )
                    xts_all[(s, ci_t)] = xf

            # bank tiles: SWDGE queue (gpsimd engine is otherwise idle)
            bts = {}
            for co_t in range(COT):
                for ci_t in range(CIT):
                    bt = bkp.tile(
                        [128, NTAP, 8, 128], bf16, tag="bank", name="bank"
                    )
                    nc.gpsimd.dma_start(
                        out=bt[:].rearrange("p a b c -> p (a b c)"),
                        in_=bank_d[co_t, ci_t],
                    )
                    bts[(co_t, ci_t)] = bt

            # gating row-sums: split across DVE and ACT
            red_scratch = smalls.tile([128, HW], bf16, tag="redscratch")
            for s in range(NS):
                for ci_t in range(CIT):
                    xf = xts_all[(s, ci_t)]
                    if (s * CIT + ci_t) % 2 == 0:
                        nc.vector.reduce_sum(
                            out=means[ci_t][:, s : s + 1],
                            in_=xf[:],
                            axis=mybir.AxisListType.X,
                        )
                    else:
                        nc.scalar.activation(
                            red_scratch[:],
                            xf[:],
                            mybir.ActivationFunctionType.Copy,
                            accum_out=means[ci_t][:, s : s + 1],
                        )

            # logits[e, s] = sum_ci fc_w[e, ci] * xsum[ci, s]
            ps_g = pp.tile([E, NS], f32, tag="ps", name="ps_g")
            for ci_t in range(CIT):
                nc.tensor.matmul(
                    ps_g[:],
                    fcwt[ci_t][:],
                    means[ci_t][:],
                    start=(ci_t == 0),
                    stop=(ci_t == CIT - 1),
                )
            gate_sb = smalls.tile([E, NS], f32, tag="gate")
            # gate = sigmoid(logits / (H*W) + fc_b)
            nc.scalar.activation(
                gate_sb[:],
                ps_g[:],
                mybir.ActivationFunctionType.Sigmoid,
                bias=fcb_sb[:],
                scale=1.0 / float(HW),
            )
            # broadcast gates to all partitions: gb[p, s] = gate[p // 16, s]
            ps_b = pp.tile([128, NS], f32, tag="ps", name="ps_b")
            nc.tensor.matmul(ps_b[:], S_sb[:], gate_sb[:], start=True, stop=True)
            gb_sb = smalls.tile([128, NS], f32, tag="gb")
            nc.scalar.copy(gb_sb[:], ps_b[:])

            # R[(e,j), (s,c')] = g[s,e] * delta(j, c')  (4 ACT scaled copies)
            rt = smalls.tile([128, NS * 16], bf16, tag="rt")
            for s in range(NS):
                nc.scalar.mul(
                    rt[:, s * 16 : (s + 1) * 16], tI_sb[:], gb_sb[:, s : s + 1]
                )

            # ------------- phase B: expert mixing on PE -------------
            wm = {}

            def emit_mix(co_t, ci_t):
                # wm[(co_t, ci_t, tap)][ci_l, s*128 + co_l] =
                #     sum_e g[s,e] * weight[e, co_t*128+co_l, ci_t*128+ci_l, tap]
                bt = bts[(co_t, ci_t)]
                for tap in range(NTAP):
                    ps_m = pp.tile([128, NS, 128], f32, tag="ps", name="ps_m")
                    for cbl in range(8):
                        nc.tensor.matmul(
                            ps_m[:, :, cbl * 16 : (cbl + 1) * 16],
                            bt[:, tap, cbl, :],
                            rt[:],
                            start=True,
                            stop=True,
                        )
                    wt = wmpool.tile([128, NS * 128], bf16, tag="wm", name="wm")
                    nc.scalar.copy(wt[:], ps_m[:].rearrange("p a b -> p (a b)"))
                    wm[(co_t, ci_t, tap)] = wt

            # ------------- phase C: conv (zero-pad via clamped MM ranges) ----
            def emit_conv_chunk(s, co_t, c, ps_c, ci_list, start, stop):
                pc3 = ps_c[:].rearrange("p (h w) -> p h w", w=W)
                first = start
                n_mm = len(ci_list) * NTAP
                i = 0
                for ci_t in ci_list:
                    xv = xts_all[(s, ci_t)][:].rearrange("p (h w) -> p h w", w=W)
                    for kh, kw in TAP_ORDER:
                        tap = kh * K + kw
                        r0, nr = 8 * c, 8
                        if c == 0 and kh == 0:
                            r0, nr = 1, 7
                        if c == NCHUNK - 1 and kh == 2:
                            nr = 7
                        dc0 = 1 if kw == 0 else 0
                        ncol = W if kw == 1 else W - 1
                        in_r0 = r0 + kh - 1
                        in_c0 = kw - 1 + dc0
                        nc.tensor.matmul(
                            pc3[:, r0 - 8 * c : r0 - 8 * c + nr, dc0 : dc0 + ncol],
                            wm[(co_t, ci_t, tap)][:, s * 128 : (s + 1) * 128],
                            xv[:, in_r0 : in_r0 + nr, in_c0 : in_c0 + ncol],
                            start=first,
                            stop=(stop and i == n_mm - 1),
                        )
                        first = False
                        i += 1

            # store slabs after chunks 1, 3, 5, 6 (16/16/16/8 rows)
            STORES = {1: (0, 16), 3: (16, 32), 5: (32, 48), 6: (48, 56)}

            def emit_conv_sample(s, co_t, skip_first_chunk=False):
                ot = op.tile([128, HW], f32, tag="outp", name="outp")
                for c in range(NCHUNK):
                    if c == 0 and skip_first_chunk:
                        ps_c = _pending_chunk[0]
                    else:
                        ps_c = pp.tile([128, CH], f32, tag="ps", name="ps_c")
                        emit_conv_chunk(s, co_t, c, ps_c, [0, 1], True, True)
                    nc.vector.tensor_copy(ot[:, c * CH : (c + 1) * CH], ps_c[:])
                    if c in STORES:
                        h0, h1 = STORES[c]
                        nc.sync.dma_start(
                            out=out_d[s, co_t * 128 : (co_t + 1) * 128, h0:h1],
                            in_=ot[:, h0 * W : h1 * W].rearrange(
                                "p (h w) -> p h w", w=W
                            ),
                        )

            # co_t = 0: interleave the first chunk's two halves with mixing so
            # the PE can start convolving before bank(0,1) has been mixed.
            emit_mix(0, 0)
            _pending_chunk = [pp0.tile([128, CH], f32, tag="ps0", name="ps_c0")]
            emit_conv_chunk(0, 0, 0, _pending_chunk[0], [0], True, False)
            emit_mix(0, 1)
            emit_conv_chunk(0, 0, 0, _pending_chunk[0], [1], False, True)
            emit_conv_sample(0, 0, skip_first_chunk=True)
            for s in range(1, NS):
                emit_conv_sample(s, 0)
            emit_mix(1, 0)
            emit_mix(1, 1)
            for s in range(NS):
                emit_conv_sample(s, 1)

    nc.compile()
    return nc


def _get_nc():
    global _CACHED_NC
    if _CACHED_NC is None:
        _CACHED_NC = _build()
    return _CACHED_NC


def _pack_bank(weight):
    # bankp[co_t, ci_t, e*16+j, ((kh*3+kw)*8 + cbl)*128 + ci_l] =
    #     weight[e*256 + co_t*128 + cbl*16 + j, ci_t*128 + ci_l, kh, kw]
    w6 = weight.reshape(E, COT, 8, 16, CIT, 128, NTAP)
    return np.ascontiguousarray(
        w6.transpose(1, 4, 0, 3, 6, 2, 5).reshape(COT, CIT, 128, NTAP * 8 * 128)
    ).astype(BF_NP)


def kernel(x, weight, fc_w, fc_b):
    assert x.shape == (N_FULL, CIN, H, W), x.shape
    assert weight.shape == (E * COUT, CIN, K, K), weight.shape
    x = np.ascontiguousarray(x, dtype=np.float32)
    weight = np.ascontiguousarray(weight, dtype=np.float32)
    fc_w = np.ascontiguousarray(fc_w, dtype=np.float32)
    fc_b = np.ascontiguousarray(fc_b, dtype=np.float32)

    xb = x.astype(BF_NP)
    bankp = _pack_bank(weight)

    nc = _get_nc()
    in_maps = [
        {
            "xb": np.ascontiguousarray(xb[i * NS : (i + 1) * NS]),
            "bankp": bankp,
            "fc_w": fc_w,
            "fc_b": fc_b,
        }
        for i in range(NCORES)
    ]
    res = run_bass_kernel_spmd(nc, in_maps, core_ids=list(range(NCORES)))
    out = np.concatenate([res.results[i]["out"] for i in range(NCORES)], axis=0)
    return out


# revision 16
# speedup vs baseline: 1.5779x; 1.0423x over previous
"""CondConv2d (moe_routing) Trainium2 Bass kernel, v3 (bf16, no padding).

Full-input contract: kernel(**inputs) takes the unsharded inputs
  x      (32, 256, 56, 56) f32
  weight (2048, 256, 3, 3) f32   -- expert bank, (E*COUT, CIN, 3, 3), E=8
  fc_w   (8, 256) f32
  fc_b   (8,) f32
and returns the full (32, 256, 56, 56) f32 output of

  gate = sigmoid(mean_hw(x) @ fc_w.T + fc_b)              # (n, 8)
  w    = (gate @ weight.reshape(8, -1)).reshape(n, 256, 256, 3, 3)
  out[s] = conv2d(x[s], w[s], padding=1)

Sharding: data-parallel over batch across 8 NeuronCores (4 samples/core),
expert bank + fc params replicated.

Host pre-pass (layout/dtype only): x and the expert bank are converted to
bf16; the bank is repacked into the expert-interleaved K-tile layout the
mixing matmuls consume (contiguous stationary slices -> FWL fast weight
load), so every DMA is a contiguous line-rate transfer.

Per-core program (heavy matmuls in bf16 = full PE rate):
  phase A  x loaded flat+contiguous, split across both HWDGE queues; the
           bank streams on the gpsimd SWDGE queue concurrently. Gating
           row-sums split across DVE (reduce_sum) and ACT (Copy with
           accum_out). Gating logits via tiny PE matmul, sigmoid on ACT,
           gates broadcast across partitions with a second tiny PE matmul,
           R built by 4 ACT scaled copies.
  phase B  expert mixing ON the PE with an expert-interleaved contraction:
           bank K-tiles have partitions (e, j) = 8 experts x 16 lanes, so
           one matmul per 16-channel group both sums over experts and
           transposes the bank into the (ci, co) lhsT layout the conv
           needs:
             out[ci, (s, co')] = sum_(e,j) bank[(e,co),ci] * R[(e,j),(s,c')]
             R[(e,j), (s,c')]  = g[s,e] * delta(j, c')
  phase C  3x3 conv as 18 accumulating bf16 matmuls (2 ci tiles x 9 taps)
           per 448-column PSUM chunk (8 output rows) DIRECTLY on the flat
           56-wide x tiles: zero-padding is realized by clamping each tap's
           matmul to the valid row/column range (the centre tap goes first
           with start=True, covering the full bank so has_written is set
           everywhere). f32 accumulate, DVE evacuation, streamed f32 stores.
"""

import numpy as np

import concourse.bass as bass
import concourse.mybir as mybir
import concourse.tile as tile
from concourse import bacc
from concourse.bass_utils import run_bass_kernel_spmd

# Problem constants (hardcoded per contract).
N_FULL = 32
NCORES = 8
NS = N_FULL // NCORES  # 4 samples per core
E = 8
CIN = 256
COUT = 256
K = 3
H = W = 56
HW = H * W  # 3136
CH = 8 * W  # 448 output columns per PSUM chunk (8 rows x 56 cols)
NCHUNK = H // 8  # 7
CIT = CIN // 128  # 2 contraction tiles
COT = COUT // 128  # 2 output-partition tiles
NTAP = K * K  # 9

# centre tap first (full range, start=True), then the edge taps
TAP_ORDER = [(1, 1), (0, 0), (0, 1), (0, 2), (1, 0), (1, 2), (2, 0), (2, 1), (2, 2)]

f32 = mybir.dt.float32
bf16 = mybir.dt.bfloat16
BF_NP = mybir.dt.np(bf16)

_CACHED_NC = None


def _build():
    nc = bacc.Bacc(trn_type="TRN2", target_bir_lowering=False, debug=False)

    x_d = nc.dram_tensor("xb", (NS, CIN, H, W), bf16, kind="ExternalInput").ap()
    # pre-packed bank: [co_t, ci_t, (e,j)=128, (tap, cbl, ci_l)=9216] bf16
    bank_d = nc.dram_tensor(
        "bankp", (COT, CIT, 128, NTAP * 8 * 128), bf16, kind="ExternalInput"
    ).ap()
    # host-transposed fc_w: fcwt[ci_t, ci_l, e] = fc_w[e, ci_t*128 + ci_l]
    fcwt_d = nc.dram_tensor("fcwt", (CIT, 128, E), f32, kind="ExternalInput").ap()
    fcb_d = nc.dram_tensor("fc_b", (E,), f32, kind="ExternalInput").ap()
    out_d = nc.dram_tensor("out", (NS, COUT, H, W), f32, kind="ExternalOutput").ap()

    # tiled 16x16 identity: tI[p, c] = 1 iff p % 16 == c
    tI_d = nc.inline_tensor(
        np.tile(np.eye(16, dtype=np.float32), (8, 1)), name="tiled_eye16"
    ).ap()
    # broadcast selector: S[e, p] = 1 iff p // 16 == e
    S_d = nc.inline_tensor(
        np.repeat(np.eye(E, dtype=np.float32), 16, axis=1), name="bcast8"
    ).ap()

    with tile.TileContext(nc) as tc:
        with (
            tc.tile_pool(name="smalls", bufs=1) as smalls,
            tc.tile_pool(name="wmix", bufs=COT * CIT * NTAP) as wmpool,
            tc.tile_pool(name="ps", bufs=7, space="PSUM") as pp,
            tc.tile_pool(name="ps0", bufs=1, space="PSUM") as pp0,
            tc.tile_pool(name="xflat", bufs=NS * CIT) as xfp,
            tc.tile_pool(name="bank", bufs=2) as bkp,
            tc.tile_pool(name="outp", bufs=2) as op,
        ):
            # ------------- phase A: DMAs first, then gating chain -------------
            # PE warmup: keep the HAM activity monitor busy during the load
            # phase so mixing + early conv run at 2.4 GHz, not 1.2.
            junk = smalls.tile([128, 512], bf16, tag="junk")
            nc.vector.memset(junk[:], 0.0)
            warm_ps = pp0.tile([128, 512], f32, tag="ps0", name="warm_ps")
            for _ in range(30):
                nc.tensor.matmul(
                    warm_ps[:], junk[:, 0:128], junk[:], start=True, stop=True
                )

            # x tiles: flat contiguous loads, 2 samples per DMA, one HWDGE
            # queue per ci_t half. xpair[(sp, ci_t)][:, sl] is sample 2sp+sl.
            xpair = {}
            for sp in range(NS // 2):
                for ci_t in range(CIT):
                    xf = xfp.tile([128, 2, HW], bf16, tag="xflat", name="xflat")
                    q = nc.sync if ci_t == 0 else nc.scalar
                    q.dma_start(
                        out=xf,
                        in_=x_d[
                            2 * sp : 2 * sp + 2, ci_t * 128 : (ci_t + 1) * 128
                        ].rearrange("s c h w -> c s (h w)"),
                    )
                    xpair[(sp, ci_t)] = xf

            def xts_all(s, ci_t):
                return xpair[(s // 2, ci_t)][:, s % 2]

            # bank tiles: SWDGE queue (gpsimd engine is otherwise idle).
            # Only 3 bufs fit in SBUF; (1,1) is loaded into (0,0)'s slot
            # once mixing has consumed it (still ~100us before it's needed).
            bts = {}

            def emit_bank_load(co_t, ci_t):
                bt = bkp.tile([128, NTAP, 8, 128], bf16, tag="bank", name="bank")
                nc.gpsimd.dma_start(
                    out=bt[:].rearrange("p a b c -> p (a b c)"),
                    in_=bank_d[co_t, ci_t],
                )
                bts[(co_t, ci_t)] = bt

            for co_t, ci_t in [(0, 0), (0, 1)]:
                emit_bank_load(co_t, ci_t)

            # small constants (after the x tiles on the scalar queue: their
            # strided descriptors would otherwise delay the bulk loads)
            means = []
            fcwt = []
            for ci_t in range(CIT):
                means.append(
                    smalls.tile([128, NS], f32, tag=f"means{ci_t}", name=f"means{ci_t}")
                )
                t = smalls.tile([128, E], f32, tag=f"fcwt{ci_t}", name=f"fcwt{ci_t}")
                nc.scalar.dma_start(out=t, in_=fcwt_d[ci_t])
                fcwt.append(t)
            fcb_sb = smalls.tile([E, 1], f32, tag="fcb")
            nc.scalar.dma_start(out=fcb_sb, in_=fcb_d.unsqueeze(1))
            tI_sb = smalls.tile([128, 16], f32, tag="tI")
            nc.scalar.dma_start(out=tI_sb, in_=tI_d)
            S_sb = smalls.tile([E, 128], f32, tag="S")
            nc.scalar.dma_start(out=S_sb, in_=S_d)

            # gating row-sums, split across DVE (ci_t=0) and ACT (ci_t=1);
            # plus a warmup-pulse matmul per tile to keep the PE HAM-warm
            # through the load phase.
            red_scratch = smalls.tile([128, HW], bf16, tag="redscratch")
            for sp in range(NS // 2):
                for ci_t in range(CIT):
                    xf = xpair[(sp, ci_t)]
                    for sl in range(2):
                        s = 2 * sp + sl
                        if ci_t == 0:
                            nc.vector.reduce_sum(
                                out=means[ci_t][:, s : s + 1],
                                in_=xf[:, sl],
                                axis=mybir.AxisListType.X,
                            )
                        else:
                            nc.scalar.activation(
                                red_scratch[:],
                                xf[:, sl],
                                mybir.ActivationFunctionType.Copy,
                                accum_out=means[ci_t][:, s : s + 1],
                            )
                    nc.tensor.matmul(
                        warm_ps[:], junk[:, 0:128], xf[:, 0, 0:512],
                        start=True, stop=True,
                    )

            # logits[e, s] = sum_ci fc_w[e, ci] * xsum[ci, s]
            ps_g = pp.tile([E, NS], f32, tag="ps", name="ps_g")
            for ci_t in range(CIT):
                nc.tensor.matmul(
                    ps_g[:],
                    fcwt[ci_t][:],
                    means[ci_t][:],
                    start=(ci_t == 0),
                    stop=(ci_t == CIT - 1),
                )
            gate_sb = smalls.tile([E, NS], f32, tag="gate")
            # gate = sigmoid(logits / (H*W) + fc_b)
            nc.scalar.activation(
                gate_sb[:],
                ps_g[:],
                mybir.ActivationFunctionType.Sigmoid,
                bias=fcb_sb[:],
                scale=1.0 / float(HW),
            )
            # broadcast gates to all partitions: gb[p, s] = gate[p // 16, s]
            ps_b = pp.tile([128, NS], f32, tag="ps", name="ps_b")
            nc.tensor.matmul(ps_b[:], S_sb[:], gate_sb[:], start=True, stop=True)
            gb_sb = smalls.tile([128, NS], f32, tag="gb")
            nc.scalar.copy(gb_sb[:], ps_b[:])

            # R[(e,j), (s,c')] = g[s,e] * delta(j, c')  (4 ACT scaled copies)
            rt = smalls.tile([128, NS * 16], bf16, tag="rt")
            for s in range(NS):
                nc.scalar.mul(
                    rt[:, s * 16 : (s + 1) * 16], tI_sb[:], gb_sb[:, s : s + 1]
                )

            # ------------- phase B: expert mixing on PE -------------
            wm = {}

            def emit_mix(co_t, ci_t):
                # wm[(co_t, ci_t, tap)][ci_l, s*128 + co_l] =
                #     sum_e g[s,e] * weight[e, co_t*128+co_l, ci_t*128+ci_l, tap]
                bt = bts[(co_t, ci_t)]
                for tap in range(NTAP):
                    ps_m = pp.tile([128, NS, 128], f32, tag="ps", name="ps_m")
                    for cbl in range(8):
                        nc.tensor.matmul(
                            ps_m[:, :, cbl * 16 : (cbl + 1) * 16],
                            bt[:, tap, cbl, :],
                            rt[:],
                            start=True,
                            stop=True,
                        )
                    wt = wmpool.tile([128, NS * 128], bf16, tag="wm", name="wm")
                    nc.scalar.copy(wt[:], ps_m[:].rearrange("p a b -> p (a b)"))
                    wm[(co_t, ci_t, tap)] = wt

            # ------------- phase C: conv (zero-pad via clamped MM ranges) ----
            def emit_conv_chunk(s, co_t, c, ps_c, ci_list, start, stop):
                pc3 = ps_c[:].rearrange("p (h w) -> p h w", w=W)
                first = start
                n_mm = len(ci_list) * NTAP
                i = 0
                for ci_t in ci_list:
                    xv = xts_all(s, ci_t).rearrange("p (h w) -> p h w", w=W)
                    for kh, kw in TAP_ORDER:
                        tap = kh * K + kw
                        r0, nr = 8 * c, 8
                        if c == 0 and kh == 0:
                            r0, nr = 1, 7
                        if c == NCHUNK - 1 and kh == 2:
                            nr = 7
                        dc0 = 1 if kw == 0 else 0
                        ncol = W if kw == 1 else W - 1
                        in_r0 = r0 + kh - 1
                        in_c0 = kw - 1 + dc0
                        nc.tensor.matmul(
                            pc3[:, r0 - 8 * c : r0 - 8 * c + nr, dc0 : dc0 + ncol],
                            wm[(co_t, ci_t, tap)][:, s * 128 : (s + 1) * 128],
                            xv[:, in_r0 : in_r0 + nr, in_c0 : in_c0 + ncol],
                            start=first,
                            stop=(stop and i == n_mm - 1),
                        )
                        first = False
                        i += 1

            # store slabs after chunks 3 and 6 (32/24 rows)
            STORES = {3: (0, 32), 6: (32, 56)}

            def emit_conv_sample(s, co_t, skip_first_chunk=False):
                ot = op.tile([128, HW], f32, tag="outp", name="outp")
                for c in range(NCHUNK):
                    if c == 0 and skip_first_chunk:
                        ps_c = _pending_chunk[0]
                    else:
                        ps_c = pp.tile([128, CH], f32, tag="ps", name="ps_c")
                        emit_conv_chunk(s, co_t, c, ps_c, [0, 1], True, True)
                    nc.vector.tensor_copy(ot[:, c * CH : (c + 1) * CH], ps_c[:])
                    if c in STORES:
                        h0, h1 = STORES[c]
                        nc.sync.dma_start(
                            out=out_d[s, co_t * 128 : (co_t + 1) * 128, h0:h1],
                            in_=ot[:, h0 * W : h1 * W].rearrange(
                                "p (h w) -> p h w", w=W
                            ),
                        )

            # co_t = 0: interleave the first chunk's two halves with mixing so
            # the PE can start convolving before bank(0,1) has been mixed.
            emit_mix(0, 0)
            emit_bank_load(1, 0)
            _pending_chunk = [pp0.tile([128, CH], f32, tag="ps0", name="ps_c0")]
            emit_conv_chunk(0, 0, 0, _pending_chunk[0], [0], True, False)
            emit_mix(0, 1)
            emit_bank_load(1, 1)
            emit_conv_chunk(0, 0, 0, _pending_chunk[0], [1], False, True)
            emit_conv_sample(0, 0, skip_first_chunk=True)
            for s in range(1, NS):
                emit_conv_sample(s, 0)
            emit_mix(1, 0)
            emit_mix(1, 1)
            for s in range(NS):
                emit_conv_sample(s, 1)

    nc.compile()
    return nc


def _get_nc():
    global _CACHED_NC
    if _CACHED_NC is None:
        _CACHED_NC = _build()
    return _CACHED_NC


def _pack_bank(weight):
    # bankp[co_t, ci_t, e*16+j, ((kh*3+kw)*8 + cbl)*128 + ci_l] =
    #     weight[e*256 + co_t*128 + cbl*16 + j, ci_t*128 + ci_l, kh, kw]
    w6 = weight.reshape(E, COT, 8, 16, CIT, 128, NTAP)
    return np.ascontiguousarray(
        w6.transpose(1, 4, 0, 3, 6, 2, 5).reshape(COT, CIT, 128, NTAP * 8 * 128)
    ).astype(BF_NP)


def kernel(x, weight, fc_w, fc_b):
    assert x.shape == (N_FULL, CIN, H, W), x.shape
    assert weight.shape == (E * COUT, CIN, K, K), weight.shape
    x = np.ascontiguousarray(x, dtype=np.float32)
    weight = np.ascontiguousarray(weight, dtype=np.float32)
    fc_w = np.ascontiguousarray(fc_w, dtype=np.float32)
    fc_b = np.ascontiguousarray(fc_b, dtype=np.float32)

    xb = x.astype(BF_NP)
    bankp = _pack_bank(weight)
    fcwt = np.ascontiguousarray(fc_w.T.reshape(CIT, 128, E))

    nc = _get_nc()
    in_maps = [
        {
            "xb": np.ascontiguousarray(xb[i * NS : (i + 1) * NS]),
            "bankp": bankp,
            "fcwt": fcwt,
            "fc_b": fc_b,
        }
        for i in range(NCORES)
    ]
    res = run_bass_kernel_spmd(nc, in_maps, core_ids=list(range(NCORES)))
    out = np.concatenate([res.results[i]["out"] for i in range(NCORES)], axis=0)
    return out
